# revision 1
# baseline (speedup 1.0000x reference)
"""Submanifold sparse conv (27-tap rulebook) + BatchNorm + ReLU on 8 trn2 cores.

Strategy:
  - Invert the scatter-add rulebook into a pure gather map g[k, j] (each
    output site has at most one input partner per offset; sentinel -> zero row).
  - Recover 3D coords of the active sites by BFS over the rulebook matchings,
    kd-median-split into 16 balanced spatial regions (2 per core) so each
    region's feature table (own rows + halo + zero row) fits int16 indices
    for dma_gather.
  - Device phase 1 (per core, per region): transpose-mode dma_gather of
    bf16 [ch|0] padded rows -> [128, n] tiles (channels on partitions),
    27 matmuls (lhsT = W[k] [Cin,Cout]) accumulate in PSUM [64, 512] fp32,
    bn_stats per tile + bn_aggr -> per-core BN stats; conv result stashed
    bf16 and written to DRAM.
  - Host combines the 8 cores' (mean, var) into global BN stats.
  - Device phase 2: out = Relu(conv * scale[c] + shift[c]) -> fp32.
  - Host scatters region rows back into the full [N, 64] output.
"""

import os
import sys

for p in ("/opt/trn_rl_repo",):
    if p not in sys.path:
        sys.path.insert(0, p)

import numpy as np
import ml_dtypes

N_ACT = 262144
C = 64
K = 27
NCORES = 8
NREG = 16
REG = N_ACT // NREG          # 16384 rows per region
TCAP = 24576                 # per-region table capacity (rows); sentinel = TCAP-1
SENT = TCAP - 1
QROWS = 4096                 # gather granularity (rows per dma_gather)
NQ = REG // QROWS            # 4 quarters per region
TPQ = QROWS // 512           # 8 psum tiles per quarter
BN_EPS = 1e-4

_OFFS = np.array([(dz, dy, dx) for dz in (-1, 0, 1) for dy in (-1, 0, 1)
                  for dx in (-1, 0, 1)], dtype=np.int32)

_cache = {}


def _build_gather_map(in_idx, out_idx):
    """g[k, j] = table row feeding output j at tap k, or -1."""
    g = np.full((K, N_ACT), -1, dtype=np.int32)
    for k in range(K):
        ii = in_idx[k]
        oo = out_idx[k]
        valid = (ii < N_ACT) & (oo < N_ACT) & (ii >= 0) & (oo >= 0)
        g[k, oo[valid]] = ii[valid]
    return g


def _recover_coords(g):
    """BFS positions from the 26 non-center matchings."""
    srcs, dsts, deltas = [], [], []
    for k in range(K):
        if k == 13:
            continue
        j = np.nonzero(g[k] >= 0)[0].astype(np.int32)
        i = g[k, j]
        srcs.append(j); dsts.append(i); deltas.append(np.broadcast_to(_OFFS[k], (len(j), 3)))
        srcs.append(i); dsts.append(j); deltas.append(np.broadcast_to(-_OFFS[k], (len(i), 3)))
    src = np.concatenate(srcs); dst = np.concatenate(dsts)
    dlt = np.concatenate(deltas).astype(np.int32)
    order = np.argsort(src, kind="stable")
    src, dst, dlt = src[order], dst[order], dlt[order]
    ptr = np.zeros(N_ACT + 1, dtype=np.int64)
    np.add.at(ptr, src + 1, 1)
    ptr = np.cumsum(ptr)

    pos = np.zeros((N_ACT, 3), dtype=np.int32)
    visited = np.zeros(N_ACT, dtype=bool)
    unseen = np.ones(N_ACT, dtype=bool)
    while True:
        seeds = np.nonzero(unseen)[0]
        if len(seeds) == 0:
            break
        s = seeds[0]
        visited[s] = True; unseen[s] = False
        frontier = np.array([s], dtype=np.int64)
        while len(frontier):
            counts = ptr[frontier + 1] - ptr[frontier]
            nz = counts > 0
            counts = counts[nz]
            starts = ptr[frontier[nz]]
            total = int(counts.sum())
            if total == 0:
                break
            # vectorized concatenation of [starts[i], starts[i]+counts[i]) ranges
            flat = np.ones(total, dtype=np.int64)
            cum = np.cumsum(counts)
            flat[0] = starts[0]
            if len(starts) > 1:
                flat[cum[:-1]] = starts[1:] - (starts[:-1] + counts[:-1]) + 1
            flat = np.cumsum(flat)
            e_dst = dst[flat]
            e_src = src[flat]
            new_mask = ~visited[e_dst]
            nd = e_dst[new_mask]
            ns = e_src[new_mask]
            ndl = dlt[flat][new_mask]
            pos[nd] = pos[ns] + ndl  # duplicate writes are consistent
            visited[nd] = True
            unseen[nd] = False
            frontier = np.unique(nd)
        # remaining unseen nodes either isolated or in other components
        # isolated (no edges): drop them from BFS loop quickly
        iso = unseen & (ptr[1:] == ptr[:-1])
        unseen[iso] = False
    return pos


def _kd_regions(pos):
    """Split sites into NREG exactly-equal regions by recursive median split."""
    ids = np.arange(N_ACT, dtype=np.int64)

    def split(ids, nleaf):
        if nleaf == 1:
            return [ids]
        spans = [pos[ids, a].max() - pos[ids, a].min() if len(ids) else 0 for a in range(3)]
        ax = int(np.argmax(spans))
        order = ids[np.argsort(pos[ids, ax], kind="stable")]
        h = len(order) // 2
        return split(order[:h], nleaf // 2) + split(order[h:], nleaf // 2)

    leaves = split(ids, NREG)
    regions = []
    for ids_r in leaves:
        key = np.lexsort((pos[ids_r, 2], pos[ids_r, 1], pos[ids_r, 0]))
        regions.append(ids_r[key])
    return regions


def _prep(features, W, in_idx, out_idx):
    g = _build_gather_map(np.asarray(in_idx), np.asarray(out_idx))
    pos = _recover_coords(g)
    regions = _kd_regions(pos)

    feats = np.asarray(features, dtype=np.float32)
    tables = np.zeros((NREG, TCAP, 128), dtype=ml_dtypes.bfloat16)
    gidx_all = np.zeros((NREG, K, 128, REG // 16), dtype=np.int16)
    lut = np.full(N_ACT + 1, -1, dtype=np.int32)
    for r, own in enumerate(regions):
        tg = g[:, own]                       # [K, REG] global targets (-1 invalid)
        valid = tg >= 0
        ext_mask = np.zeros(N_ACT, dtype=bool)
        ext_mask[tg[valid]] = True
        ext_mask[own] = False
        halo = np.nonzero(ext_mask)[0]
        n_ids = len(own) + len(halo)
        assert n_ids <= SENT, f"region {r}: table {n_ids} > {SENT}"
        table_ids = np.concatenate([own, halo])
        lut[:] = -1
        lut[table_ids] = np.arange(n_ids, dtype=np.int32)
        tgs = np.where(valid, tg, N_ACT)
        loc = lut[tgs]
        loc = np.where(loc < 0, SENT, loc).astype(np.int16)   # [K, REG]
        tables[r, :n_ids, :C] = feats[table_ids].astype(ml_dtypes.bfloat16)
        # wrap 16 + replicate 8x
        w = loc.reshape(K, REG // 16, 16).transpose(0, 2, 1)  # [K, 16, REG/16]
        gidx_all[r] = np.tile(w, (1, 8, 1))
    wT = np.ascontiguousarray(np.asarray(W, dtype=np.float32).transpose(1, 0, 2)
                              ).astype(ml_dtypes.bfloat16)    # [Cin, K, Cout]
    return g, pos, regions, tables, gidx_all, wT


# ----------------------------------------------------------------------------
# device kernels
# ----------------------------------------------------------------------------

def _build_phase1():
    import concourse.bass as bass
    import concourse.tile as tile
    from concourse import bacc, mybir, library_config
    from contextlib import ExitStack

    f32 = mybir.dt.float32
    bf16 = mybir.dt.bfloat16
    i16 = mybir.dt.int16

    nc = bacc.Bacc("TRN2", target_bir_lowering=False, debug=False,
                   num_devices=NCORES)
    table_d = nc.dram_tensor("table", [2, TCAP, 128], bf16, kind="ExternalInput")
    gidx_d = nc.dram_tensor("gidx", [2, K, 128, REG // 16], i16, kind="ExternalInput")
    w_d = nc.dram_tensor("w", [C, K, C], bf16, kind="ExternalInput")
    stash_d = nc.dram_tensor("stash", [2, C, REG], bf16, kind="ExternalOutput")
    stats_d = nc.dram_tensor("stats", [C, 2], f32, kind="ExternalOutput")

    with ExitStack() as ctx:
        tc = ctx.enter_context(tile.TileContext(nc))
        singles = ctx.enter_context(tc.tile_pool(name="singles", bufs=1))
        gbufs = ctx.enter_context(tc.tile_pool(name="gbufs", bufs=4))
        ibufs = ctx.enter_context(tc.tile_pool(name="ibufs", bufs=4))
        psums = ctx.enter_context(tc.tile_pool(name="psum", bufs=8, space="PSUM"))
        stbufs = ctx.enter_context(tc.tile_pool(name="stbufs", bufs=4))

        nc.gpsimd.load_library(library_config.mlp)

        w_sb = singles.tile([C, K, C], bf16, name="w_sb", tag="w_sb")
        nc.sync.dma_start(w_sb[:], w_d[:])
        stats_sb = singles.tile([C, 2 * NQ * TPQ, 6], f32, name="stats_sb", tag="stats_sb")

        ntile = 0
        for r in range(2):
            for q in range(NQ):
                pt = [psums.tile([C, 512], f32, name="pt", tag="pt") for _ in range(TPQ)]
                for k in range(K):
                    it = ibufs.tile([128, QROWS // 16], i16, name="it", tag="it")
                    nc.sync.dma_start(
                        it[:], gidx_d[r, k, :, q * (QROWS // 16):(q + 1) * (QROWS // 16)])
                    gb = gbufs.tile([128, 1, QROWS], bf16, name="gb", tag="gb")
                    nc.gpsimd.dma_gather(gb[:], table_d[r], it[:], QROWS, QROWS,
                                         128, transpose=True,
                                         single_packet=False)
                    for t in range(TPQ):
                        nc.tensor.matmul(
                            out=pt[t][:],
                            lhsT=w_sb[:, k, :],
                            rhs=gb[0:C, 0, t * 512:(t + 1) * 512],
                            start=(k == 0), stop=(k == K - 1),
                            skip_group_check=True)
                sb = stbufs.tile([C, QROWS], bf16, name="sb", tag="sb")
                for t in range(TPQ):
                    nc.vector.bn_stats(out=stats_sb[:, ntile, :], in_=pt[t][:])
                    nc.vector.tensor_copy(out=sb[:, t * 512:(t + 1) * 512],
                                          in_=pt[t][:])
                    ntile += 1
                nc.sync.dma_start(stash_d[r, :, q * QROWS:(q + 1) * QROWS], sb[:])

        mv = singles.tile([C, 2], f32, name="mv", tag="mv")
        nc.vector.bn_aggr(out=mv[:], in_=stats_sb[:])
        nc.sync.dma_start(stats_d[:], mv[:])
    nc.compile()
    return nc


def _build_phase2():
    import concourse.tile as tile
    from concourse import bacc, mybir
    from contextlib import ExitStack

    f32 = mybir.dt.float32
    bf16 = mybir.dt.bfloat16

    nc = bacc.Bacc("TRN2", target_bir_lowering=False, debug=False,
                   num_devices=NCORES)
    stash_d = nc.dram_tensor("stash", [2, C, REG], bf16, kind="ExternalInput")
    ss_d = nc.dram_tensor("ss", [C, 2], f32, kind="ExternalInput")
    out_d = nc.dram_tensor("out", [2, C, REG], f32, kind="ExternalOutput")

    with ExitStack() as ctx:
        tc = ctx.enter_context(tile.TileContext(nc))
        singles = ctx.enter_context(tc.tile_pool(name="singles", bufs=1))
        bufs = ctx.enter_context(tc.tile_pool(name="bufs", bufs=3))
        obufs = ctx.enter_context(tc.tile_pool(name="obufs", bufs=3))

        ss_sb = singles.tile([C, 2], f32, name="ss_sb", tag="ss_sb")
        nc.sync.dma_start(ss_sb[:], ss_d[:])
        for r in range(2):
            for q in range(NQ):
                xb = bufs.tile([C, QROWS], bf16, name="xb", tag="xb")
                nc.sync.dma_start(xb[:], stash_d[r, :, q * QROWS:(q + 1) * QROWS])
                ob = obufs.tile([C, QROWS], f32, name="ob", tag="ob")
                nc.scalar.activation(
                    out=ob[:], in_=xb[:],
                    func=mybir.ActivationFunctionType.Relu,
                    bias=ss_sb[:, 1:2], scale=ss_sb[:, 0:1])
                nc.sync.dma_start(out_d[r, :, q * QROWS:(q + 1) * QROWS], ob[:])
    nc.compile()
    return nc


def _get_kernels():
    if "k1" not in _cache:
        _cache["k1"] = _build_phase1()
        _cache["k2"] = _build_phase2()
    return _cache["k1"], _cache["k2"]


def _run_device(tables, gidx_all, wT, gamma, beta, trace=False):
    from concourse import bass_utils

    k1, k2 = _get_kernels()
    in_maps1 = []
    for c in range(NCORES):
        in_maps1.append({
            "table": np.ascontiguousarray(tables[2 * c:2 * c + 2]),
            "gidx": np.ascontiguousarray(gidx_all[2 * c:2 * c + 2]),
            "w": wT,
        })
    res1 = bass_utils.run_bass_kernel_spmd(k1, in_maps1, core_ids=list(range(NCORES)),
                                           trace=trace)
    t1 = res1.exec_time_ns

    # combine per-core stats (equal counts per core)
    means = np.stack([r["stats"][:, 0] for r in res1.results])   # [8, 64]
    varis = np.stack([r["stats"][:, 1] for r in res1.results])
    gmean = means.mean(axis=0)
    gex2 = (varis + means * means).mean(axis=0)
    gvar = gex2 - gmean * gmean
    rstd = 1.0 / np.sqrt(gvar + BN_EPS)
    scale = (np.asarray(gamma, np.float64) * rstd).astype(np.float32)
    shift = (np.asarray(beta, np.float64) - gmean * np.asarray(gamma, np.float64) * rstd
             ).astype(np.float32)
    ss = np.stack([scale, shift], axis=1).astype(np.float32)     # [64, 2]

    in_maps2 = [{"stash": res1.results[c]["stash"], "ss": ss} for c in range(NCORES)]
    res2 = bass_utils.run_bass_kernel_spmd(k2, in_maps2, core_ids=list(range(NCORES)),
                                           trace=trace)
    t2 = res2.exec_time_ns
    outs = [res2.results[c]["out"] for c in range(NCORES)]       # [2, 64, REG] each
    return outs, (t1, t2)


def _emulate_device(tables, gidx_all, wT, gamma, beta):
    """Numpy emulation of exactly what the device computes (bf16 matmuls)."""
    wf = np.asarray(wT, dtype=np.float32)        # [Cin, K, Cout]
    outs = []
    sums = np.zeros((NREG, C), np.float64)
    sqs = np.zeros((NREG, C), np.float64)
    convs = []
    for r in range(NREG):
        tab = np.asarray(tables[r], np.float32)[:, :C]           # [TCAP, 64]
        acc = np.zeros((REG, C), np.float32)
        for k in range(K):
            w = gidx_all[r, k, :16, :]                            # [16, REG/16]
            loc = w.T.reshape(-1).astype(np.int64)                # unwrap
            acc += tab[loc] @ wf[:, k, :]
        accb = acc.astype(ml_dtypes.bfloat16).astype(np.float32)  # stash rounding
        convs.append(accb)
        sums[r] = acc.sum(0)
        sqs[r] = (acc.astype(np.float64) ** 2).sum(0)
    gmean = sums.sum(0) / N_ACT
    gvar = sqs.sum(0) / N_ACT - gmean ** 2
    rstd = 1.0 / np.sqrt(gvar + BN_EPS)
    scale = np.asarray(gamma, np.float64) * rstd
    shift = np.asarray(beta, np.float64) - gmean * scale
    for r in range(NREG):
        o = np.maximum(convs[r] * scale + shift, 0).astype(np.float32)
        outs.append(o)
    return outs


def kernel(features, W, gamma, beta, in_idx, out_idx, _trace=False, _emulate=False):
    g, pos, regions, tables, gidx_all, wT = _prep(features, W, in_idx, out_idx)
    gamma = np.asarray(gamma, np.float32)
    beta = np.asarray(beta, np.float32)

    out_full = np.zeros((N_ACT, C), dtype=np.float32)
    if _emulate:
        regs = _emulate_device(tables, gidx_all, wT, gamma, beta)
        for r in range(NREG):
            out_full[regions[r]] = regs[r]
        return out_full

    outs, times = _run_device(tables, gidx_all, wT, gamma, beta, trace=_trace)
    for c in range(NCORES):
        for rr in range(2):
            r = 2 * c + rr
            out_full[regions[r]] = outs[c][rr].T.astype(np.float32)
    kernel.last_times = times
    return out_full



# revision 3
# speedup vs baseline: 1.0339x; 1.0339x over previous
"""Submanifold sparse conv (27-tap rulebook) + BN + ReLU on 8 trn2 cores, v2.

Compacted-rulebook design (vs baseline's full-tile gather):
  k1: per (region, tap): dma_gather ONLY the valid partner rows (ch-major),
      conv with W[k] into packed psum columns, PE-transpose the packed
      [64, n] result to site-major [n, 64] fp16, dma_scatter_add the rows
      into a per-region fp16 accumulator in DRAM at true output positions.
      Valid pairs are ~32% of K*N -> gather descriptors (the cost
      bottleneck) drop ~3x vs the full-tile baseline.
  k2a: per-core BN stats from acc via gram matmuls (sum & sum-of-squares).
  host: combine stats across cores -> scale/shift vectors.
  k2b: site-major scale+shift+relu (free-dim broadcast), fp32 rows out.
"""

import sys

for p in ("/opt/trn_rl_repo",):
    if p not in sys.path:
        sys.path.insert(0, p)

import numpy as np
import ml_dtypes

N_ACT = 262144
C = 64
K = 27
NCORES = 8
NREG = 16
REG = N_ACT // NREG          # 16384 rows per region
TCAP = 24576                 # per-region table capacity; SENT = zero row
SENT = TCAP - 1
ACC_ROWS = REG + 128         # + dummy rows for stream padding
DUMMY = REG                  # scatter dest for padding slots
BN_EPS = 1e-4
NGRP = 7                     # tap groups per region (gather/scatter granularity)

_OFFS = np.array([(dz, dy, dx) for dz in (-1, 0, 1) for dy in (-1, 0, 1)
                  for dx in (-1, 0, 1)], dtype=np.int32)

_cache = {}
_IDENT = np.eye(C, dtype=ml_dtypes.float16 if hasattr(ml_dtypes, 'float16') else np.float16)


# --------------------------------------------------------------------------
# host prep (rulebook compilation)
# --------------------------------------------------------------------------

def _build_gather_map(in_idx, out_idx):
    g = np.full((K, N_ACT), -1, dtype=np.int32)
    for k in range(K):
        ii = in_idx[k]
        oo = out_idx[k]
        valid = (ii < N_ACT) & (oo < N_ACT) & (ii >= 0) & (oo >= 0)
        g[k, oo[valid]] = ii[valid]
    return g


def _recover_coords(g):
    srcs, dsts, deltas = [], [], []
    for k in range(K):
        if k == 13:
            continue
        j = np.nonzero(g[k] >= 0)[0].astype(np.int32)
        i = g[k, j]
        srcs.append(j); dsts.append(i); deltas.append(np.broadcast_to(_OFFS[k], (len(j), 3)))
        srcs.append(i); dsts.append(j); deltas.append(np.broadcast_to(-_OFFS[k], (len(i), 3)))
    src = np.concatenate(srcs); dst = np.concatenate(dsts)
    dlt = np.concatenate(deltas).astype(np.int32)
    order = np.argsort(src, kind="stable")
    src, dst, dlt = src[order], dst[order], dlt[order]
    ptr = np.zeros(N_ACT + 1, dtype=np.int64)
    np.add.at(ptr, src + 1, 1)
    ptr = np.cumsum(ptr)

    pos = np.zeros((N_ACT, 3), dtype=np.int32)
    visited = np.zeros(N_ACT, dtype=bool)
    unseen = np.ones(N_ACT, dtype=bool)
    while True:
        seeds = np.nonzero(unseen)[0]
        if len(seeds) == 0:
            break
        s = seeds[0]
        visited[s] = True; unseen[s] = False
        frontier = np.array([s], dtype=np.int64)
        while len(frontier):
            counts = ptr[frontier + 1] - ptr[frontier]
            nz = counts > 0
            counts = counts[nz]
            starts = ptr[frontier[nz]]
            total = int(counts.sum())
            if total == 0:
                break
            flat = np.ones(total, dtype=np.int64)
            cum = np.cumsum(counts)
            flat[0] = starts[0]
            if len(starts) > 1:
                flat[cum[:-1]] = starts[1:] - (starts[:-1] + counts[:-1]) + 1
            flat = np.cumsum(flat)
            e_dst = dst[flat]
            e_src = src[flat]
            new_mask = ~visited[e_dst]
            nd = e_dst[new_mask]
            ns = e_src[new_mask]
            ndl = dlt[flat][new_mask]
            pos[nd] = pos[ns] + ndl
            visited[nd] = True
            unseen[nd] = False
            frontier = np.unique(nd)
        iso = unseen & (ptr[1:] == ptr[:-1])
        unseen[iso] = False
    return pos


def _kd_regions(pos):
    ids = np.arange(N_ACT, dtype=np.int64)

    def split(ids, nleaf):
        if nleaf == 1:
            return [ids]
        spans = [pos[ids, a].max() - pos[ids, a].min() if len(ids) else 0 for a in range(3)]
        ax = int(np.argmax(spans))
        order = ids[np.argsort(pos[ids, ax], kind="stable")]
        h = len(order) // 2
        return split(order[:h], nleaf // 2) + split(order[h:], nleaf // 2)

    leaves = split(ids, NREG)
    regions = []
    for ids_r in leaves:
        key = np.lexsort((pos[ids_r, 2], pos[ids_r, 1], pos[ids_r, 0]))
        regions.append(ids_r[key])
    return regions


def _wrap16(idx):
    """[n] -> [128, n//16] int16 (16-wrap, replicated 8x across partition groups)."""
    n = len(idx)
    assert n % 16 == 0
    w = idx.reshape(n // 16, 16).T                      # [16, n/16]
    return np.tile(w, (8, 1)).astype(np.int16)


def _prep(features, W, in_idx, out_idx):
    g = _build_gather_map(np.asarray(in_idx), np.asarray(out_idx))
    pos = _recover_coords(g)
    regions = _kd_regions(pos)

    feats = np.asarray(features, dtype=np.float32)
    tables = np.zeros((NREG, TCAP, 128), dtype=ml_dtypes.bfloat16)
    lut = np.full(N_ACT, -1, dtype=np.int32)

    # per (region, tap) compacted (src, dst) streams
    raw = [[None] * K for _ in range(NREG)]
    for r, own in enumerate(regions):
        tg = g[:, own]                        # [K, REG]
        valid = tg >= 0
        ext_mask = np.zeros(N_ACT, dtype=bool)
        ext_mask[tg[valid]] = True
        ext_mask[own] = False
        halo = np.nonzero(ext_mask)[0]
        n_ids = len(own) + len(halo)
        assert n_ids <= SENT, f"region {r}: table {n_ids} > {SENT}"
        table_ids = np.concatenate([own, halo])
        lut[:] = -1
        lut[table_ids] = np.arange(n_ids, dtype=np.int32)
        tables[r, :n_ids, :C] = feats[table_ids].astype(ml_dtypes.bfloat16)
        for k in range(K):
            j = np.nonzero(valid[k])[0]                 # dst local rows
            src = lut[tg[k, j]]
            raw[r][k] = (src.astype(np.int32), j.astype(np.int32))

    # common padded size per (slot, tap): max over cores for that region slot
    mp = np.zeros((2, K), dtype=np.int64)
    for r in range(NREG):
        s = r % 2
        for k in range(K):
            m = len(raw[r][k][0])
            mp[s, k] = max(mp[s, k], (m + 127) & ~127)

    # greedy-balance taps into NGRP groups per slot (same grouping both slots)
    load = mp.sum(axis=0)
    order = np.argsort(-load)
    gsum = np.zeros(NGRP, dtype=np.int64)
    groups = [[] for _ in range(NGRP)]
    for k in order:
        gi = int(np.argmin(gsum))
        groups[gi].append(int(k))
        gsum[gi] += load[k]
    segplan = []          # [slot][gi] -> list of (k, mp)
    gsizes = []           # [slot][gi] -> stream length
    for s in range(2):
        sp, gs = [], []
        for gi in range(NGRP):
            sp.append([(k, int(mp[s, k])) for k in groups[gi]])
            gs.append(int(sum(mp[s, k] for k in groups[gi])))
        segplan.append(sp)
        gsizes.append(gs)

    # per-core index arrays, common layout
    NT16 = max(sum(gsizes[0]), sum(gsizes[1])) // 16
    gidx = np.full((NCORES, 2, 128, NT16), SENT, dtype=np.int16)
    sidx = np.full((NCORES, 2, 128, NT16), DUMMY, dtype=np.int16)
    for c in range(NCORES):
        for s in range(2):
            r = 2 * c + s
            off = 0
            for gi in range(NGRP):
                for (k, mpk) in segplan[s][gi]:
                    src, dst = raw[r][k]
                    m = len(src)
                    sa = np.full(mpk, SENT, np.int32)
                    da = np.full(mpk, DUMMY, np.int32)
                    sa[:m] = src
                    da[:m] = dst
                    gidx[c, s, :, off // 16:(off + mpk) // 16] = _wrap16(sa)
                    sidx[c, s, :, off // 16:(off + mpk) // 16] = _wrap16(da)
                    off += mpk

    wT = np.ascontiguousarray(np.asarray(W, dtype=np.float32).transpose(1, 0, 2)
                              ).astype(ml_dtypes.bfloat16)    # [Cin, K, Cout]
    return regions, tables, segplan, gsizes, NT16, gidx, sidx, wT


# --------------------------------------------------------------------------
# device kernels
# --------------------------------------------------------------------------

def _build_k1(segplan, gsizes, NT16):
    import concourse.tile as tile
    from concourse import bacc, mybir, library_config
    from contextlib import ExitStack

    f32 = mybir.dt.float32
    f16 = mybir.dt.float16
    bf16 = mybir.dt.bfloat16
    i16 = mybir.dt.int16

    nc = bacc.Bacc("TRN2", target_bir_lowering=False, debug=False,
                   num_devices=NCORES)
    table_d = nc.dram_tensor("table", [2, TCAP, 128], bf16, kind="ExternalInput")
    gidx_d = nc.dram_tensor("gidx", [2, 128, NT16], i16, kind="ExternalInput")
    sidx_d = nc.dram_tensor("sidx", [2, 128, NT16], i16, kind="ExternalInput")
    w_d = nc.dram_tensor("w", [C, K, C], bf16, kind="ExternalInput")
    id_d = nc.dram_tensor("ident", [C, C], f16, kind="ExternalInput")
    acc_d = nc.dram_tensor("acc", [2, ACC_ROWS, 128], f16, kind="ExternalOutput")

    with ExitStack() as ctx:
        tc = ctx.enter_context(tile.TileContext(nc))
        singles = ctx.enter_context(tc.tile_pool(name="singles", bufs=1))
        gbufs = ctx.enter_context(tc.tile_pool(name="gbufs", bufs=2))
        ibufs = ctx.enter_context(tc.tile_pool(name="ibufs", bufs=2))
        sibufs = ctx.enter_context(tc.tile_pool(name="sibufs", bufs=2))
        d1bufs = ctx.enter_context(tc.tile_pool(name="d1bufs", bufs=4))
        s2bufs = ctx.enter_context(tc.tile_pool(name="s2bufs", bufs=2))
        psums = ctx.enter_context(tc.tile_pool(name="psum", bufs=3, space="PSUM"))
        psum2s = ctx.enter_context(tc.tile_pool(name="psum2", bufs=3, space="PSUM"))

        nc.gpsimd.load_library(library_config.mlp)

        w_sb = singles.tile([C, K, C], bf16, name="w_sb", tag="w_sb")
        nc.sync.dma_start(w_sb[:], w_d[:])
        ident = singles.tile([C, C], f16, name="ident", tag="ident")
        nc.sync.dma_start(ident[:], id_d[:])

        # zero-init acc (deps to the scatters are tracked via shadow memory)
        zt = singles.tile([128, 2064], f16, name="zt", tag="zt")
        nc.vector.memset(zt[:], 0.0)
        acc_flat = acc_d[:].rearrange("a b c -> (a b c)")
        ZN = 128 * 2064
        for z in range(2 * ACC_ROWS * 128 // ZN):
            nc.sync.dma_start(acc_flat[z * ZN:(z + 1) * ZN], zt[:])

        drain_eng = [nc.vector, nc.scalar]
        di = 0
        for s in range(2):
            off = 0
            for gi in range(NGRP):
                n = gsizes[s][gi]
                it = ibufs.tile([128, n // 16], i16, name="it", tag="it")
                nc.sync.dma_start(it[:], gidx_d[s, :, off // 16:(off + n) // 16])
                gb = gbufs.tile([128, 1, n], bf16, name="gb", tag="gb")
                GC = 8192
                for c0 in range(0, n, GC):
                    cn = min(GC, n - c0)
                    nc.gpsimd.dma_gather(
                        gb[:, :, c0:c0 + cn], table_d[s],
                        it[:, c0 // 16:(c0 + cn) // 16], cn, cn, 128,
                        transpose=True, single_packet=False)

                st = sibufs.tile([128, n // 16], i16, name="st", tag="st")
                nc.sync.dma_start(st[:], sidx_d[s, :, off // 16:(off + n) // 16])
                s2 = s2bufs.tile([128, n // 128, C], f16, name="s2", tag="s2")

                pos = 0
                for (k, mpk) in segplan[s][gi]:
                    for c0 in range(0, mpk, 512):
                        nn = min(512, mpk - c0)
                        pt = psums.tile([C, 512], f32, name="pt", tag="pt")
                        nc.tensor.matmul(
                            out=pt[:, 0:nn],
                            lhsT=w_sb[:, k, :],
                            rhs=gb[0:C, 0, pos + c0:pos + c0 + nn],
                            start=True, stop=True,
                            skip_group_check=True)
                        d1 = d1bufs.tile([C, 512], f16, name="d1", tag="d1")
                        eng = drain_eng[di % 2]
                        if eng is nc.vector:
                            eng.tensor_copy(out=d1[:, 0:nn], in_=pt[:, 0:nn])
                        else:
                            eng.copy(out=d1[:, 0:nn], in_=pt[:, 0:nn])
                        p2 = psum2s.tile([128, 512], f16, name="p2", tag="p2")
                        nb = nn // 128
                        for b in range(nb):
                            nc.tensor.transpose(
                                p2[:, b * C:(b + 1) * C],
                                d1[:, b * 128:(b + 1) * 128],
                                ident[:])
                        eng2 = drain_eng[(di + 1) % 2]
                        dst = s2[:, (pos + c0) // 128:(pos + c0) // 128 + nb, :]
                        src = p2[:, 0:nb * C].rearrange("p (x c) -> p x c", c=C)
                        if eng2 is nc.vector:
                            eng2.tensor_copy(out=dst, in_=src)
                        else:
                            eng2.copy(out=dst, in_=src)
                        di += 1
                    pos += mpk
                SC = 6144
                for c0 in range(0, n, SC):
                    cn = min(SC, n - c0)
                    nc.gpsimd.dma_scatter_add(
                        out_ap=acc_d[s, :, 0:C],
                        in_ap=s2[:, c0 // 128:(c0 + cn) // 128, :],
                        idxs_ap=st[:, c0 // 16:(c0 + cn) // 16],
                        num_idxs=cn, num_idxs_reg=cn, elem_size=C,
                        elem_step=128, single_packet=False)
                off += n
    nc.compile()
    return nc


def _build_k2a():
    """Per-core BN partial stats from acc: psum += chunk^T @ chunk and ^T @ 1."""
    import concourse.tile as tile
    from concourse import bacc, mybir
    from contextlib import ExitStack

    f32 = mybir.dt.float32
    f16 = mybir.dt.float16

    nc = bacc.Bacc("TRN2", target_bir_lowering=False, debug=False,
                   num_devices=NCORES)
    acc_d = nc.dram_tensor("acc", [2, ACC_ROWS, 128], f16, kind="ExternalInput")
    stats_d = nc.dram_tensor("stats", [C, C + 1], f32, kind="ExternalOutput")

    NCH = 32              # 128-row chunks per load tile
    with ExitStack() as ctx:
        tc = ctx.enter_context(tile.TileContext(nc))
        singles = ctx.enter_context(tc.tile_pool(name="singles", bufs=1))
        abufs = ctx.enter_context(tc.tile_pool(name="abufs", bufs=3))
        psums = ctx.enter_context(tc.tile_pool(name="psum", bufs=1, space="PSUM"))

        ones = singles.tile([128, 1], f16, name="ones", tag="ones")
        nc.vector.memset(ones[:], 1.0)
        ps = psums.tile([C, C + 1], f32, name="ps", tag="ps")
        nsteps = 2 * (REG // (128 * NCH))
        step = 0
        for r in range(2):
            for c0 in range(0, REG, 128 * NCH):
                ab = abufs.tile([128, NCH, C], f16, name="ab", tag="ab")
                nc.sync.dma_start(
                    ab[:],
                    acc_d[r, c0:c0 + 128 * NCH, 0:C].rearrange(
                        "(x p) c -> p x c", p=128))
                for x in range(NCH):
                    first = step == 0 and x == 0
                    last = step == nsteps - 1 and x == NCH - 1
                    nc.tensor.matmul(out=ps[:, 0:C], lhsT=ab[:, x, :],
                                     rhs=ab[:, x, :], start=first, stop=last,
                                     skip_group_check=True)
                    nc.tensor.matmul(out=ps[:, C:C + 1], lhsT=ab[:, x, :],
                                     rhs=ones[:], start=first, stop=last,
                                     skip_group_check=True)
                step += 1
        out_sb = singles.tile([C, C + 1], f32, name="out_sb", tag="out_sb")
        nc.vector.tensor_copy(out=out_sb[:], in_=ps[:])
        nc.sync.dma_start(stats_d[:], out_sb[:])
    nc.compile()
    return nc


def _build_k2b():
    """Apply scale/shift/relu in site-major layout; write fp32 rows."""
    import concourse.tile as tile
    from concourse import bacc, mybir
    from contextlib import ExitStack

    f32 = mybir.dt.float32
    f16 = mybir.dt.float16

    nc = bacc.Bacc("TRN2", target_bir_lowering=False, debug=False,
                   num_devices=NCORES)
    acc_d = nc.dram_tensor("acc", [2, ACC_ROWS, 128], f16, kind="ExternalInput")
    ss_d = nc.dram_tensor("ss", [128, 2 * C], f32, kind="ExternalInput")
    out_d = nc.dram_tensor("out", [2, REG, C], f32, kind="ExternalOutput")

    NCH = 16
    with ExitStack() as ctx:
        tc = ctx.enter_context(tile.TileContext(nc))
        singles = ctx.enter_context(tc.tile_pool(name="singles", bufs=1))
        abufs = ctx.enter_context(tc.tile_pool(name="abufs", bufs=3))
        obufs = ctx.enter_context(tc.tile_pool(name="obufs", bufs=3))

        ss = singles.tile([128, 2 * C], f32, name="ss", tag="ss")
        nc.sync.dma_start(ss[:], ss_d[:])
        sc = ss[:, 0:C].unsqueeze(1).broadcast_to([128, NCH, C])
        sh = ss[:, C:2 * C].unsqueeze(1).broadcast_to([128, NCH, C])
        for r in range(2):
            for c0 in range(0, REG, 128 * NCH):
                ab = abufs.tile([128, NCH, C], f16, name="ab", tag="ab")
                nc.sync.dma_start(
                    ab[:],
                    acc_d[r, c0:c0 + 128 * NCH, 0:C].rearrange(
                        "(p x) c -> p x c", p=128))
                ob = obufs.tile([128, NCH, C], f32, name="ob", tag="ob")
                nc.vector.tensor_tensor(out=ob[:], in0=ab[:], in1=sc,
                                        op=mybir.AluOpType.mult)
                nc.vector.tensor_tensor(out=ob[:], in0=ob[:], in1=sh,
                                        op=mybir.AluOpType.add)
                nc.scalar.activation(out=ob[:], in_=ob[:],
                                     func=mybir.ActivationFunctionType.Relu)
                nc.sync.dma_start(out_d[r, c0:c0 + 128 * NCH, :], ob[:])
    nc.compile()
    return nc


# --------------------------------------------------------------------------
# runner
# --------------------------------------------------------------------------

def _get_kernels(segplan, gsizes, NT16):
    key = ("k", NT16, tuple(gsizes[0]), tuple(gsizes[1]))
    if key not in _cache:
        _cache.clear()
        _cache[key] = (_build_k1(segplan, gsizes, NT16), _build_k2a(), _build_k2b())
    return _cache[key]


def kernel(features, W, gamma, beta, in_idx, out_idx, _trace=False):
    from concourse import bass_utils

    regions, tables, segplan, gsizes, NT16, gidx, sidx, wT = _prep(
        features, W, in_idx, out_idx)
    gamma = np.asarray(gamma, np.float64)
    beta = np.asarray(beta, np.float64)

    k1, k2a, k2b = _get_kernels(segplan, gsizes, NT16)

    in1 = []
    for c in range(NCORES):
        in1.append({
            "table": np.ascontiguousarray(tables[2 * c:2 * c + 2]),
            "gidx": np.ascontiguousarray(gidx[c]),
            "sidx": np.ascontiguousarray(sidx[c]),
            "w": wT,
            "ident": _IDENT,
        })
    res1 = bass_utils.run_bass_kernel_spmd(k1, in1, core_ids=list(range(NCORES)),
                                           trace=_trace)
    accs = [res1.results[c]["acc"] for c in range(NCORES)]

    in2a = [{"acc": accs[c]} for c in range(NCORES)]
    res2a = bass_utils.run_bass_kernel_spmd(k2a, in2a, core_ids=list(range(NCORES)))

    s1 = np.zeros(C, np.float64)
    s2 = np.zeros(C, np.float64)
    for c in range(NCORES):
        st = np.asarray(res2a.results[c]["stats"], np.float64)
        s1 += st[:, C]
        s2 += np.diag(st[:, 0:C])
    gmean = s1 / N_ACT
    gvar = s2 / N_ACT - gmean * gmean
    rstd = 1.0 / np.sqrt(gvar + BN_EPS)
    scale = (gamma * rstd).astype(np.float32)
    shift = (beta - gmean * gamma * rstd).astype(np.float32)
    ss = np.zeros((128, 2 * C), np.float32)
    ss[:, 0:C] = scale[None, :]
    ss[:, C:2 * C] = shift[None, :]

    in2b = [{"acc": accs[c], "ss": ss} for c in range(NCORES)]
    res2b = bass_utils.run_bass_kernel_spmd(k2b, in2b, core_ids=list(range(NCORES)))

    out_full = np.zeros((N_ACT, C), dtype=np.float32)
    for c in range(NCORES):
        o = res2b.results[c]["out"]                        # [2, REG, C]
        for s in range(2):
            out_full[regions[2 * c + s]] = o[s]
    kernel.last_kernels = (k1, k2a, k2b)
    return out_full
